# revision 1
# baseline (speedup 1.0000x reference)
"""LSTM decoder with attention (image captioning) — Trainium2 Bass kernel.

Sharding: data-parallel over batch (64 images -> 8 cores x 8 images).
The whole per-step recurrence is collective-free; host does cheap glue
(embedding gather, weight transposes, h0/c0 init, final bias add).

Device program per core (b = 8 local images):
  pre:   enc_projT[a, (b,j,q)] = wenc @ IF.T      (attention projection)
         IFW[(b,j,q), d4]      = IF @ Wc.T        (image features pre-multiplied
                                                   by the LSTM input weights)
  loop (t = 0..19, serial):
         hprojT = wdec @ h          -> tanh(enc_projT + hprojT) -> e = V . att
         softmax(e) -> w  -> transpose w (PE)
         gates_T = W_hh@h + embproj[t] + sum_p w[b,p] * IFW[b,p,:]   (PSUM accum)
         LSTM cell elementwise (transposed layout) -> h written into H_T
  tail:  logits = H_T.T @ fc_w.T    (one big matmul over all 20 steps)

All recurrence matmuls are bf16 (FWL fast weight loads); accumulation fp32.
"""

import os
import sys
import numpy as np

for _p in ("/opt/trn_rl_repo",):
    if _p not in sys.path and os.path.isdir(_p):
        sys.path.insert(0, _p)

import ml_dtypes  # noqa: E402

import concourse.bass as bass  # noqa: E402
import concourse.tile as tile  # noqa: E402
from concourse import bacc, mybir  # noqa: E402
from concourse.bass import ts  # noqa: E402
from concourse.bass_utils import run_bass_kernel_spmd  # noqa: E402

AF = mybir.ActivationFunctionType
F32 = mybir.dt.float32
BF16 = mybir.dt.bfloat16
BF = ml_dtypes.bfloat16

# problem shapes (hardcoded)
VOCAB, ENC, EMB, DEC, ATT = 10000, 2048, 512, 512, 512
B, P, S = 64, 196, 20
NCORES = 8
NB = B // NCORES          # 8 images per core
PPAD = 256                # P padded to 2 k-tiles per image
NJ = PPAD // 128          # 2
NBJ = NB * NJ             # 16 (b,j) k-tiles
NE = ENC // 128           # 16
NA = ATT // 128           # 4
ND = DEC // 128           # 4
NG = (4 * DEC) // 128     # 16 gate tiles (i 0-3, f 4-7, g 8-11, o 12-15)
D4 = 4 * DEC              # 2048
NVC = 20                  # vocab chunks
VC = VOCAB // NVC         # 500

_CACHE = {}
TRACE = False  # set by test.py to capture an NTFF profile


def _build_nc():
    if "nc" in _CACHE:
        return _CACHE["nc"]

    nc = bacc.Bacc(
        "TRN2",
        target_bir_lowering=False,
        debug=False,
        enable_asserts=False,
        num_devices=NCORES,
    )

    def din(name, shape, dt=BF16):
        return nc.dram_tensor(name, shape, dt, kind="ExternalInput").ap()

    ift_d = din("ift", [NE, 128, NB * PPAD])        # IF.T  [e, (b,j,q)] padded
    wct_d = din("wct", [NE, 128, D4])               # Wc.T  [e, d4]
    wenct_d = din("wenct", [NE, 128, ATT])          # wenc.T [e, a]
    whht_d = din("whht", [ND, 128, D4])             # W_hh.T [dec, d4]
    wdect_d = din("wdect", [ND, 128, ATT])          # wdec.T [dec, a]
    vt_d = din("vt", [NA, 128, 1])                  # V_w.T
    ept_d = din("ept", [128, S * NG * NB])          # embprojT [r, (t, m, b)]
    i128_d = din("i128", [128, 128])                # identity bf16
    fct_d = din("fct", [ND, 128, VOCAB])            # fc_w.T [dec, vocab]
    h0t_d = din("h0t", [ND, 128, NB], F32)
    c0t_d = din("c0t", [ND, 128, NB], F32)
    encb_d = din("encb", [NA, 128, 1], F32)
    wdecb_d = din("wdecb", [NA, 128, 1], F32)
    out_d = nc.dram_tensor("out", [S * NB, VOCAB], F32, kind="ExternalOutput").ap()

    with tile.TileContext(nc) as tc:
        from contextlib import ExitStack

        with ExitStack() as glob_ctx:
            gp = glob_ctx.enter_context(tc.tile_pool(name="glob", bufs=1))
            # persistent state / loop constants
            ifw = [gp.tile([128, D4], BF16, name=f"ifw{i}", tag=f"ifw{i}")
                   for i in range(NBJ)]
            encp = [gp.tile([128, NB * PPAD], BF16, name=f"encp{i}", tag=f"encp{i}")
                    for i in range(NA)]
            HT = [gp.tile([128, (S + 1) * NB], F32, name=f"HT{i}", tag=f"HT{i}")
                  for i in range(ND)]
            cT = [gp.tile([128, NB], F32, name=f"cT{i}", tag=f"cT{i}")
                  for i in range(ND)]
            i128 = gp.tile([128, 128], BF16, name="i128")
            vt = [gp.tile([128, 1], BF16, name=f"vt{i}", tag=f"vt{i}")
                  for i in range(NA)]
            encb = [gp.tile([128, 1], F32, name=f"encb{i}", tag=f"encb{i}")
                    for i in range(NA)]
            wdecb = [gp.tile([128, 1], F32, name=f"wdecb{i}", tag=f"wdecb{i}")
                     for i in range(NA)]
            ones_col = gp.tile([128, 1], F32, name="ones_col")
            ones_row = gp.tile([1, 128], F32, name="ones_row")
            hp_sb = [gp.tile([128, NB], F32, name=f"hp{i}", tag=f"hp{i}")
                     for i in range(NA)]
            et_exp = [gp.tile([128, NB], F32, name=f"etx{j}", tag=f"etx{j}")
                      for j in range(NJ)]
            rsum_sb = gp.tile([1, NB], F32, name="rsum_sb")
            h_bf = [gp.tile([128, NB], BF16, name=f"hbf{i}", tag=f"hbf{i}")
                    for i in range(ND)]
            wt_sb = [gp.tile([128, NB], BF16, name=f"wt{j}", tag=f"wt{j}")
                     for j in range(NJ)]
            whht = [gp.tile([128, D4], BF16, name=f"whht{k}", tag=f"whht{k}")
                    for k in range(ND)]
            wdect = [gp.tile([128, ATT], BF16, name=f"wdect{k}",
                             tag=f"wdect{k}") for k in range(ND)]
            ept = gp.tile([128, S * NG * NB], BF16, name="ept")

            nc.sync.dma_start(out=i128, in_=i128_d)
            nc.sync.dma_start(out=ept, in_=ept_d)
            for k in range(ND):
                nc.sync.dma_start(out=whht[k], in_=whht_d[k])
                nc.sync.dma_start(out=wdect[k], in_=wdect_d[k])
            for i in range(NA):
                nc.sync.dma_start(out=vt[i], in_=vt_d[i])
                nc.sync.dma_start(out=encb[i], in_=encb_d[i])
                nc.sync.dma_start(out=wdecb[i], in_=wdecb_d[i])
            for k in range(ND):
                nc.sync.dma_start(out=HT[k][:, 0:NB], in_=h0t_d[k])
                nc.sync.dma_start(out=cT[k], in_=c0t_d[k])
            nc.vector.memset(ones_col, 1.0)
            nc.vector.memset(ones_row, 1.0)

            # ---------------- pre-loop: enc_proj and IFW ----------------
            with tc.tile_pool(name="pre", bufs=1) as pre:
                ift = [pre.tile([128, NB * PPAD], BF16, name=f"ift{k}",
                                tag=f"ift{k}") for k in range(NE)]
                for k in range(NE):
                    nc.sync.dma_start(out=ift[k], in_=ift_d[k])

                with tc.tile_pool(name="wen", bufs=1) as wen, \
                     tc.tile_pool(name="pspre", bufs=4, space="PSUM") as pspre:
                    wenct = [wen.tile([128, ATT], BF16, name=f"wen{k}",
                                      tag=f"wen{k}") for k in range(NE)]
                    for k in range(NE):
                        nc.sync.dma_start(out=wenct[k], in_=wenct_d[k])
                    # enc_projT [a, (b,j,q)] += wenc_b
                    for i in range(NA):
                        for c in range(NB * PPAD // 512):
                            ps = pspre.tile([128, 512], F32, name="eps", tag="mm")
                            for k in range(NE):
                                nc.tensor.matmul(
                                    ps, wenct[k][:, ts(i, 128)],
                                    ift[k][:, ts(c, 512)],
                                    start=(k == 0), stop=(k == NE - 1))
                            nc.vector.tensor_scalar_add(
                                encp[i][:, ts(c, 512)], ps, encb[i])

                # IFW = IF @ Wc.T, in two d4 halves to bound SBUF
                for half in range(2):
                    with tc.tile_pool(name=f"wc{half}", bufs=1) as wcp, \
                         tc.tile_pool(name=f"psw{half}", bufs=4,
                                      space="PSUM") as psw:
                        wch = [wcp.tile([128, 1024], BF16, name=f"wc{k}",
                                        tag=f"wc{k}") for k in range(NE)]
                        for k in range(NE):
                            nc.sync.dma_start(
                                out=wch[k],
                                in_=wct_d[k][:, half * 1024:(half + 1) * 1024])
                        for bj in range(NBJ):
                            for c in range(2):
                                ps = psw.tile([128, 512], F32, name="wps",
                                              tag="mm")
                                for k in range(NE):
                                    nc.tensor.matmul(
                                        ps, ift[k][:, ts(bj, 128)],
                                        wch[k][:, ts(c, 512)],
                                        start=(k == 0), stop=(k == NE - 1))
                                dst = ifw[bj][:, half * 1024 + c * 512:
                                              half * 1024 + (c + 1) * 512]
                                if (bj + c) % 2 == 0:
                                    nc.scalar.copy(out=dst, in_=ps)
                                else:
                                    nc.vector.tensor_copy(out=dst, in_=ps)

            # ---------------- recurrence ----------------
            with tc.tile_pool(name="att", bufs=1) as ap_, \
                 tc.tile_pool(name="psl", bufs=1, space="PSUM") as psl, \
                 tc.tile_pool(name="cw", bufs=2) as cw:
                att = [ap_.tile([128, NB * PPAD], BF16, name=f"att{i}",
                                tag=f"att{i}") for i in range(NA)]
                # zero the padded att columns once: q in [P-128, 128) of the
                # j=1 block reads them in the e matmuls
                for i in range(NA):
                    for b in range(NB):
                        nc.vector.memset(att[i][:, b * PPAD + P:(b + 1) * PPAD],
                                         0.0)
                for t in range(S):
                    hof = t * NB  # h_t column offset in HT
                    for k in range(ND):
                        nc.scalar.copy(out=h_bf[k], in_=HT[k][:, hof:hof + NB])
                    # hprojT = wdec @ h + wdec_b
                    for i in range(NA):
                        ps = psl.tile([128, NB], F32, name="hps", tag="pa",
                                      bufs=2)
                        for k in range(ND):
                            nc.tensor.matmul(ps, wdect[k][:, ts(i, 128)],
                                             h_bf[k], start=(k == 0),
                                             stop=(k == ND - 1))
                        nc.vector.tensor_scalar_add(hp_sb[i], ps, wdecb[i])
                    # att = tanh(enc + hproj); e_T[q, b] = V . att  (transposed)
                    etp = []
                    for j in range(NJ):
                        etp.append(psl.tile([128, NB], F32, name="etp",
                                            tag="pa", bufs=2))
                    for b in range(NB):
                        lo = b * PPAD
                        for i in range(NA):
                            nc.scalar.activation(
                                att[i][:, lo:lo + P], encp[i][:, lo:lo + P],
                                AF.Tanh, bias=hp_sb[i][:, b:b + 1])
                        for j in range(NJ):
                            for i in range(NA):
                                nc.tensor.matmul(
                                    etp[j][:, b:b + 1],
                                    att[i][:, lo + j * 128:lo + (j + 1) * 128],
                                    vt[i], start=(i == 0), stop=(i == NA - 1),
                                    skip_group_check=True)
                    # softmax over P without max-subtraction (|e| <= sum|V| ~ 11,
                    # exp stays well inside fp32 range)
                    for j in range(NJ):
                        nc.scalar.activation(et_exp[j], etp[j], AF.Exp)
                    sum_ps = psl.tile([1, NB], F32, name="sum_ps", tag="sm",
                                      bufs=2)
                    nc.tensor.matmul(sum_ps, ones_col[0:128], et_exp[0],
                                     start=True, stop=False,
                                     skip_group_check=True)
                    nc.tensor.matmul(sum_ps, ones_col[0:P - 128],
                                     et_exp[1][0:P - 128, :],
                                     start=False, stop=True,
                                     skip_group_check=True)
                    nc.vector.reciprocal(rsum_sb, sum_ps)
                    rs_ps = psl.tile([128, NB], F32, name="rs_ps", tag="sm",
                                     bufs=2)
                    nc.tensor.matmul(rs_ps, ones_row, rsum_sb,
                                     start=True, stop=True)
                    for j in range(NJ):
                        nc.vector.tensor_mul(wt_sb[j], et_exp[j], rs_ps)
                    # gates (transposed): W_hh@h + embproj + attention context
                    for r in range(ND):
                        gps = []
                        for gate in range(4):
                            m = gate * 4 + r
                            gp_ = psl.tile([128, NB], F32, name="gps", tag="g",
                                           bufs=4)
                            for k in range(ND):
                                nc.tensor.matmul(
                                    gp_, whht[k][:, ts(m, 128)], h_bf[k],
                                    start=(k == 0), stop=False,
                                    skip_group_check=True)
                            ec = (t * NG + m) * NB
                            nc.tensor.matmul(gp_, i128, ept[:, ec:ec + NB],
                                             start=False, stop=False,
                                             skip_group_check=True)
                            for b in range(NB):
                                for j in range(NJ):
                                    nc.tensor.matmul(
                                        gp_[:, b:b + 1],
                                        ifw[b * NJ + j][:, ts(m, 128)],
                                        wt_sb[j][:, b:b + 1],
                                        start=False, stop=(j == NJ - 1),
                                        skip_group_check=True)
                            gps.append(gp_)
                        si = cw.tile([128, NB], F32, name="si", tag="si")
                        sf = cw.tile([128, NB], F32, name="sf", tag="sf")
                        tg = cw.tile([128, NB], F32, name="tg", tag="tg")
                        so = cw.tile([128, NB], F32, name="so", tag="so")
                        nc.scalar.activation(si, gps[0], AF.Sigmoid)
                        nc.scalar.activation(sf, gps[1], AF.Sigmoid)
                        nc.scalar.activation(tg, gps[2], AF.Tanh)
                        nc.scalar.activation(so, gps[3], AF.Sigmoid)
                        t1 = cw.tile([128, NB], F32, name="t1", tag="t1")
                        t2 = cw.tile([128, NB], F32, name="t2", tag="t2")
                        nc.vector.tensor_mul(t1, sf, cT[r])
                        nc.vector.tensor_mul(t2, si, tg)
                        nc.vector.tensor_add(cT[r], t1, t2)
                        th = cw.tile([128, NB], F32, name="th", tag="th")
                        nc.scalar.activation(th, cT[r], AF.Tanh)
                        nc.vector.tensor_mul(HT[r][:, hof + NB:hof + 2 * NB],
                                             so, th)

            # ---------------- tail: logits ----------------
            with tc.tile_pool(name="fc", bufs=1) as fcp, \
                 tc.tile_pool(name="pst", bufs=4, space="PSUM") as pst:
                fct = [fcp.tile([128, VOCAB], BF16, name=f"fct{k}",
                                tag=f"fct{k}") for k in range(ND)]
                Hb = [fcp.tile([128, S * NB], BF16, name=f"Hb{k}",
                               tag=f"Hb{k}") for k in range(ND)]
                for k in range(ND):
                    nc.sync.dma_start(out=fct[k], in_=fct_d[k])
                    nc.scalar.copy(out=Hb[k], in_=HT[k][:, NB:(S + 1) * NB])
                for mi, (m0, msz) in enumerate(((0, 128), (128, S * NB - 128))):
                    for c in range(NVC):
                        ps = pst.tile([128, VC], F32, name="lps", tag="l")
                        for k in range(ND):
                            nc.tensor.matmul(
                                ps[:msz], Hb[k][:, m0:m0 + msz],
                                fct[k][:, ts(c, VC)],
                                start=(k == 0), stop=(k == ND - 1))
                        lg = fcp.tile([128, VC], F32, name="lg", tag="lg",
                                      bufs=4)
                        if c % 2 == 0:
                            nc.scalar.copy(out=lg[:msz], in_=ps[:msz])
                        else:
                            nc.vector.tensor_copy(out=lg[:msz], in_=ps[:msz])
                        nc.sync.dma_start(
                            out=out_d[m0:m0 + msz, ts(c, VC)], in_=lg[:msz])

    nc.compile()
    _CACHE["nc"] = nc
    return nc


def _prep_core_inputs(image_feat, embproj, h0, c0, wct, wenct, whht, wdect,
                      vt, i128, fct, encb, wdecb, core):
    bs = slice(core * NB, (core + 1) * NB)
    ifp = np.zeros((NB, PPAD, ENC), np.float32)
    ifp[:, :P, :] = image_feat[bs]
    # [e, (b, j, q)]
    ift = np.ascontiguousarray(
        ifp.reshape(NB * PPAD, ENC).T).astype(BF).reshape(NE, 128, NB * PPAD)
    ep = embproj[bs]                                   # [8, 20, 2048]
    ept = np.ascontiguousarray(
        ep.transpose(2, 1, 0)                          # [2048, 20, 8]
        .reshape(NG, 128, S, NB)                       # [m, r, t, b]
        .transpose(1, 2, 0, 3)                         # [r, t, m, b]
        .reshape(128, S * NG * NB)).astype(BF)
    h0t = np.ascontiguousarray(h0[bs].T).reshape(ND, 128, NB).astype(np.float32)
    c0t = np.ascontiguousarray(c0[bs].T).reshape(ND, 128, NB).astype(np.float32)
    return dict(ift=ift, wct=wct, wenct=wenct, whht=whht, wdect=wdect, vt=vt,
                ept=ept, i128=i128, fct=fct, h0t=h0t, c0t=c0t, encb=encb,
                wdecb=wdecb)


def kernel(image_feat, captions_ids, wenc_w, wenc_b, wdec_w, wdec_b,
           V_w, V_b, embed_w, h0_w, h0_b, c0_w, c0_b,
           W_ih, b_ih, W_hh, b_hh, fc_w, fc_b):
    image_feat = np.asarray(image_feat, np.float32)
    ids = np.asarray(captions_ids).astype(np.int64)

    # host-side glue (cheap, not on the device critical path)
    emb_seq = np.asarray(embed_w, np.float32)[ids]            # [B, S, EMB]
    We = np.asarray(W_ih, np.float32)[:, ENC:]                # [D4, EMB]
    Wc = np.asarray(W_ih, np.float32)[:, :ENC]                # [D4, ENC]
    embproj = emb_seq @ We.T + (np.asarray(b_ih) + np.asarray(b_hh))
    avg = image_feat.mean(axis=1)
    h0 = np.maximum(avg @ np.asarray(h0_w, np.float32).T + h0_b, 0.0)
    c0 = np.maximum(avg @ np.asarray(c0_w, np.float32).T + c0_b, 0.0)

    wct = np.ascontiguousarray(Wc.T).astype(BF).reshape(NE, 128, D4)
    wenct = np.ascontiguousarray(
        np.asarray(wenc_w, np.float32).T).astype(BF).reshape(NE, 128, ATT)
    whht = np.ascontiguousarray(
        np.asarray(W_hh, np.float32).T).astype(BF).reshape(ND, 128, D4)
    wdect = np.ascontiguousarray(
        np.asarray(wdec_w, np.float32).T).astype(BF).reshape(ND, 128, ATT)
    vtt = np.ascontiguousarray(
        np.asarray(V_w, np.float32)[0]).astype(BF).reshape(NA, 128, 1)
    i128 = np.eye(128, dtype=BF)
    fct = np.ascontiguousarray(
        np.asarray(fc_w, np.float32).T).astype(BF).reshape(ND, 128, VOCAB)
    encb = np.asarray(wenc_b, np.float32).reshape(NA, 128, 1)
    wdecb = np.asarray(wdec_b, np.float32).reshape(NA, 128, 1)

    nc = _build_nc()
    in_maps = [
        _prep_core_inputs(image_feat, embproj, h0, c0, wct, wenct, whht,
                          wdect, vtt, i128, fct, encb, wdecb, c)
        for c in range(NCORES)
    ]
    res = run_bass_kernel_spmd(nc, in_maps, core_ids=list(range(NCORES)),
                               trace=TRACE)
    if TRACE:
        _CACHE["last_results"] = res

    preds = np.empty((B, S, VOCAB), np.float32)
    for c in range(NCORES):
        lg = res.results[c]["out"].reshape(S, NB, VOCAB)
        preds[c * NB:(c + 1) * NB] = lg.transpose(1, 0, 2)
    preds += np.asarray(fc_b, np.float32)
    return preds


if __name__ == "__main__":
    sys.path.insert(0, os.path.dirname(os.path.abspath(__file__)))
    import reference

    inputs = reference.setup_inputs()
    inputs = {k: np.asarray(v) for k, v in inputs.items()}
    expected = np.asarray(reference.reference(**inputs))
    actual = kernel(**inputs)
    err = np.abs(actual - expected)
    rel = np.linalg.norm(actual - expected) / np.linalg.norm(expected)
    print("max abs err:", err.max(), "rel:", rel)

